# revision 7
# baseline (speedup 1.0000x reference)
"""DeepAR (2-layer LSTM, H=512) Trainium2 Bass kernel.

Full-input contract: kernel(**inputs) takes the unsharded inputs from
setup_inputs() and returns the full [512, 64, 2] output.  Internally the
batch (512) is sharded 64-per-core across 8 NeuronCores (data parallel);
LSTM weights are replicated.

Device strategy (per core, B=64):
  - Weights resident in SBUF, streamed through the PE as the MOVING
    matmul operand (float32r, N=512 -> ~150ns/MM incl. weight load).
  - Gates accumulate in PSUM as paired [64, 1024] tiles: IF = (i|f),
    GO = (g|o).  One sigmoid covers i and f; composite tiles make the
    cell update 3 DVE ops: M = sigmoid(IF) * (g|c);  c' = M_l + M_r;
    h = sigmoid(o) * tanh(c').
  - L1 bias (and +1 forget bias) folds in via the ones-row of the xy
    feature chunk; L2 bias is injected as a K=1 ones x b2-row matmul
    that opens each PSUM accumulation group (no vector-engine cost).
  - h is transposed back to [H, B] via PE transpose; transpose / head
    PSUM shares banks with gate tiles whose reads retire early
    (all 8 PSUM banks are gate storage).
  - h2's o*tanh(c) runs on GpSimd (Pool) to offload DVE.
  - Autoregressive decode feeds m = h2 @ Wm + bm back into the feature
    row in-place in SBUF; mean/disp outputs accumulate in the same tile.
"""
import sys

sys.path.insert(0, "/opt/trn_rl_repo")

import numpy as np

import concourse.bass as bass
import concourse.mybir as mybir
from concourse import bass_utils, tile

F32 = mybir.dt.float32
F32R = mybir.dt.float32r
Act = mybir.ActivationFunctionType

B_FULL, TP, TO, F, H = 512, 192, 128, 64, 512
NC = 8
B = B_FULL // NC            # 64 per core
G = 4 * H                   # 2048 gate width
NSLOT = TP + 1              # 193 feature slots (slot t feeds step t)
XCOLS = NSLOT * B           # 12352

# gate column ranges in the 2048-wide haiku order (i, g, f, o)
GI, GG, GF, GO = 0, H, 2 * H, 3 * H


def ts(i, n):
    return slice(i * n, (i + 1) * n)


def split_excess_waits(nc):
    """Walrus accepts only one sync-wait per hardware instruction. Hoist
    excess waits onto NoOps (same engine) inserted right before."""
    n = 0
    for f in nc.m.functions:
        for blk in f.blocks:
            out = []
            for inst in blk.instructions:
                si = inst.sync_info
                if si is not None and si.on_wait and len(si.on_wait) > 1:
                    waits = list(si.on_wait)
                    for j, w in enumerate(waits[:-1]):
                        nop = mybir.InstNoOp(
                            name=f"{inst.name}-wnop{j}", ins=[], outs=[])
                        nop.engine = inst.engine
                        nop.sync_info = mybir.SyncInfo(on_wait=[w], on_update=[])
                        out.append(nop)
                        n += 1
                    inst.sync_info = mybir.SyncInfo(
                        on_wait=[waits[-1]], on_update=list(si.on_update))
                out.append(inst)
            blk.instructions = out
    return n


def build_program(tp=TP, to=TO, split_waits=True):
    NSLOT_ = tp + 1
    XCOLS_ = NSLOT_ * B
    nc = bass.Bass("TRN2", target_bir_lowering=False, debug=False,
                   num_devices=NC)

    xyf_d = nc.dram_tensor("xyf_d", [66, XCOLS_], F32R, kind="ExternalInput").ap()
    w1c0_d = nc.dram_tensor("w1c0_d", [66, G], F32R, kind="ExternalInput").ap()
    w1h_d = nc.dram_tensor("w1h_d", [128, 4 * G], F32R, kind="ExternalInput").ap()
    w2_d = nc.dram_tensor("w2_d", [128, 8 * G], F32R, kind="ExternalInput").ap()
    wmd_d = nc.dram_tensor("wmd_d", [128, 4 * 64], F32R, kind="ExternalInput").ap()
    b2row_d = nc.dram_tensor("b2row_d", [1, G], F32R, kind="ExternalInput").ap()
    bmd_d = nc.dram_tensor("bmd_d", [33, 1], F32, kind="ExternalInput").ap()
    id_d = nc.dram_tensor("id_d", [64, 64], F32, kind="ExternalInput").ap()
    out_d = nc.dram_tensor("out_d", [2, (tp - to) * B], F32,
                           kind="ExternalOutput").ap()

    with tile.TileContext(nc) as tc:
        with tc.sbuf_pool(name="const", bufs=1) as cp, \
             tc.sbuf_pool(name="work", bufs=1) as wp, \
             tc.psum_pool(name="ps", bufs=1) as pp:
            # ---- persistent tiles + input DMA ----
            xyf = cp.tile([66, XCOLS_], F32R, name="xyf")
            w1c0 = cp.tile([66, G], F32R, name="w1c0")
            w1h = cp.tile([128, 4 * G], F32R, name="w1h")
            w2 = cp.tile([128, 8 * G], F32R, name="w2")
            wmd = cp.tile([128, 4 * 64], F32R, name="wmd")
            b2row = cp.tile([1, G], F32R, name="b2row")
            bmd = cp.tile([33, 1], F32, name="bmd")
            ident = cp.tile([64, 64], F32, name="ident")

            nc.sync.dma_start(xyf[:, :], xyf_d[:, :])
            nc.sync.dma_start(w1c0[:, :], w1c0_d[:, :])
            for k in range(4):
                nc.sync.dma_start(w1h[:, ts(k, G)], w1h_d[:, ts(k, G)])
            for k in range(8):
                nc.sync.dma_start(w2[:, ts(k, G)], w2_d[:, ts(k, G)])
            nc.sync.dma_start(wmd[:, :], wmd_d[:, :])
            nc.sync.dma_start(b2row[:, :], b2row_d[:, :])
            nc.sync.dma_start(bmd[:, :], bmd_d[:, :])
            nc.sync.dma_start(ident[:, :], id_d[:, :])

            # ---- state composites: CC = [g | c] per layer ----
            CC1 = cp.tile([64, 2 * H], F32, name="CC1")
            CC2 = cp.tile([64, 2 * H], F32, name="CC2")
            nc.vector.memset(CC1[:, :], 0.0)
            nc.vector.memset(CC2[:, :], 0.0)

            ones_t = cp.tile([1, 64], F32, name="ones_t")
            nc.vector.memset(ones_t[:, :], 1.0)
            ONES = ones_t[:, :].bitcast(F32R)

            def lstm_post(ps_if, ps_go, CC, htag):
                """paired gate psums -> h [64, H] sbuf tile (fp32)."""
                # g -> CC left half; (i|f) -> SX; o -> Os
                nc.scalar.activation(CC[:, 0:H], ps_go[:, 0:H], Act.Tanh)
                sx = wp.tile([64, 2 * H], F32, name=f"sx{htag}",
                             tag=f"sx{htag}")
                nc.scalar.activation(sx[:, :], ps_if[:, :], Act.Sigmoid)
                os_t = wp.tile([64, H], F32, name=f"os{htag}", tag=f"os{htag}")
                nc.scalar.activation(os_t[:, :], ps_go[:, H:2 * H], Act.Sigmoid)
                # M = (i|f) * (g|c) ; c' = M_l + M_r ; h = o * tanh(c')
                m_t = wp.tile([64, 2 * H], F32, name=f"m{htag}", tag=f"m{htag}")
                nc.vector.tensor_mul(m_t[:, :], sx[:, :], CC[:, :])
                nc.vector.tensor_add(CC[:, H:2 * H], m_t[:, 0:H],
                                     m_t[:, H:2 * H])
                tc_s = wp.tile([64, H], F32, name=f"tc{htag}", tag=f"tc{htag}")
                nc.scalar.activation(tc_s[:, :], CC[:, H:2 * H], Act.Tanh)
                h = wp.tile([64, H], F32, name=f"h{htag}", tag=f"h{htag}")
                nc.vector.tensor_mul(h[:, :], os_t[:, :], tc_s[:, :])
                return h

            def transpose_h(h, trp, htag):
                """h [64,512] -> 4 hT chunk tiles [128,64]; chunk k usable as
                soon as its own transpose+copy retire (pipelines into MMs)."""
                chunks = []
                for kk in range(4):
                    nc.tensor.transpose(trp[:, ts(kk, 64)],
                                        h[:, ts(kk, 128)], ident[:, :])
                for kk in range(4):
                    hTk = wp.tile([128, 64], F32R, name=f"hT{htag}k{kk}",
                                  tag=f"hT{htag}k{kk}", bufs=2)
                    nc.vector.tensor_copy(hTk[:, :], trp[:, ts(kk, 64)])
                    chunks.append(hTk)
                return chunks

            h1T = None
            h2T_prev = None
            ps1_pend = None   # (ps_if, ps_go) accumulating L1 gates for t

            for t in range(tp):
                first = t == 0
                # --- L2 psum groups open with the bias matmul (pure fill) ---
                ps2_if = pp.tile([64, 2 * H], F32, name="ps2if", tag="A2",
                                 bufs=1)
                ps2_go = pp.tile([64, 2 * H], F32, name="ps2go", tag="B2",
                                 bufs=1)
                nc.tensor.matmul(ps2_if[:, 0:H], ONES, b2row[:, GI:GI + H],
                                 start=True, stop=False, skip_group_check=True)
                nc.tensor.matmul(ps2_if[:, H:2 * H], ONES, b2row[:, GF:GF + H],
                                 start=True, stop=False, skip_group_check=True)
                nc.tensor.matmul(ps2_go[:, 0:H], ONES, b2row[:, GG:GG + H],
                                 start=True, stop=False, skip_group_check=True)
                nc.tensor.matmul(ps2_go[:, H:2 * H], ONES, b2row[:, GO:GO + H],
                                 start=True, stop=False, skip_group_check=True)
                # --- L2 h2-part (needs h2T(t-1)) ---
                if not first:
                    for k in range(4):
                        wk = (4 + k) * G
                        st = h2T_prev[k][:, :]
                        nc.tensor.matmul(ps2_go[:, 0:H], st,
                                         w2[:, wk + GG:wk + GG + H],
                                         start=False, stop=False,
                                         skip_group_check=True)
                        nc.tensor.matmul(ps2_go[:, H:2 * H], st,
                                         w2[:, wk + GO:wk + GO + H],
                                         start=False, stop=False,
                                         skip_group_check=True)
                        nc.tensor.matmul(ps2_if[:, 0:H], st,
                                         w2[:, wk + GI:wk + GI + H],
                                         start=False, stop=False,
                                         skip_group_check=True)
                        nc.tensor.matmul(ps2_if[:, H:2 * H], st,
                                         w2[:, wk + GF:wk + GF + H],
                                         start=False, stop=False,
                                         skip_group_check=True)
                # --- L1(t): finish gates with the xy chunk (go first) ---
                if ps1_pend is None:
                    ps1_if = pp.tile([64, 2 * H], F32, name="ps1if", tag="A1",
                                     bufs=1)
                    ps1_go = pp.tile([64, 2 * H], F32, name="ps1go", tag="B1",
                                     bufs=1)
                else:
                    ps1_if, ps1_go = ps1_pend
                xs = xyf[0:66, ts(t, 64)]
                nc.tensor.matmul(ps1_go[:, 0:H], xs, w1c0[:, GG:GG + H],
                                 start=first, stop=True, skip_group_check=True)
                nc.tensor.matmul(ps1_go[:, H:2 * H], xs, w1c0[:, GO:GO + H],
                                 start=first, stop=True, skip_group_check=True)
                nc.tensor.matmul(ps1_if[:, 0:H], xs, w1c0[:, GI:GI + H],
                                 start=first, stop=True, skip_group_check=True)
                nc.tensor.matmul(ps1_if[:, H:2 * H], xs, w1c0[:, GF:GF + H],
                                 start=first, stop=True, skip_group_check=True)
                # --- L1 post + h1 transpose (trp shares B1's bank space) ---
                h1 = lstm_post(ps1_if, ps1_go, CC1, "1")
                trp1 = pp.tile([128, 256], F32, name="trp1", tag="B1", bufs=1)
                h1T = transpose_h(h1, trp1, "1")
                # --- L2 h1-part (go first; closes L2 groups) ---
                for k in range(4):
                    wk = k * G
                    st = h1T[k][:, :]
                    nc.tensor.matmul(ps2_go[:, 0:H], st,
                                     w2[:, wk + GG:wk + GG + H],
                                     start=False, stop=(k == 3),
                                     skip_group_check=True)
                    nc.tensor.matmul(ps2_go[:, H:2 * H], st,
                                     w2[:, wk + GO:wk + GO + H],
                                     start=False, stop=(k == 3),
                                     skip_group_check=True)
                    nc.tensor.matmul(ps2_if[:, 0:H], st,
                                     w2[:, wk + GI:wk + GI + H],
                                     start=False, stop=(k == 3),
                                     skip_group_check=True)
                    nc.tensor.matmul(ps2_if[:, H:2 * H], st,
                                     w2[:, wk + GF:wk + GF + H],
                                     start=False, stop=(k == 3),
                                     skip_group_check=True)
                # --- L1(t+1) h-part (pipelined ahead) ---
                if t < tp - 1:
                    nif = pp.tile([64, 2 * H], F32, name="ps1ifn", tag="A1",
                                  bufs=1)
                    ngo = pp.tile([64, 2 * H], F32, name="ps1gon", tag="B1",
                                  bufs=1)
                    for k in range(4):
                        wk = k * G
                        st = h1T[k][:, :]
                        nc.tensor.matmul(ngo[:, 0:H], st,
                                         w1h[:, wk + GG:wk + GG + H],
                                         start=(k == 0), stop=False,
                                         skip_group_check=True)
                        nc.tensor.matmul(ngo[:, H:2 * H], st,
                                         w1h[:, wk + GO:wk + GO + H],
                                         start=(k == 0), stop=False,
                                         skip_group_check=True)
                        nc.tensor.matmul(nif[:, 0:H], st,
                                         w1h[:, wk + GI:wk + GI + H],
                                         start=(k == 0), stop=False,
                                         skip_group_check=True)
                        nc.tensor.matmul(nif[:, H:2 * H], st,
                                         w1h[:, wk + GF:wk + GF + H],
                                         start=(k == 0), stop=False,
                                         skip_group_check=True)
                    ps1_pend = (nif, ngo)
                else:
                    ps1_pend = None
                # --- L2 post + h2 transpose (trp2 shares B2's bank space) ---
                h2 = lstm_post(ps2_if, ps2_go, CC2, "2")
                trp2 = pp.tile([128, 256], F32, name="trp2", tag="B2", bufs=1)
                h2T = transpose_h(h2, trp2, "2")
                # --- m/d head (AR feedback + outputs) ---
                if t >= to - 1:
                    mdp = pp.tile([64, 64], F32, name="mdp", tag="B2", bufs=1)
                    for k in range(4):
                        nc.tensor.matmul(mdp[:, :], wmd[:, ts(k, 64)],
                                         h2T[k][:, :], start=(k == 0),
                                         stop=(k == 3),
                                         skip_group_check=True)
                    # m -> feature row 0, slot t+1 (f32r rounding on write)
                    nc.scalar.activation(xyf[0:1, ts(t + 1, 64)],
                                         mdp[0:1, :], Act.Identity,
                                         bias=bmd[0:1, 0:1], scale=1.0)
                    if t >= to:
                        # d -> row 64 (ones/d row), slot t (already consumed)
                        nc.scalar.activation(xyf[64:65, ts(t, 64)],
                                             mdp[32:33, :], Act.Identity,
                                             bias=bmd[32:33, 0:1], scale=1.0)
                h2T_prev = h2T

            # ---- outputs: mean row = slots TO+1..TP, disp row = slots TO..TP-1
            nc.sync.dma_start(out_d[0:1, :],
                              xyf[0:1, (to + 1) * B:(tp + 1) * B].bitcast(F32))
            nc.sync.dma_start(out_d[1:2, :],
                              xyf[64:65, to * B:tp * B].bitcast(F32))

    n = split_excess_waits(nc) if split_waits else 0
    return nc, n


_CACHE = {}


def _get_program():
    if "nc" not in _CACHE:
        _CACHE["nc"] = build_program()[0]
    return _CACHE["nc"]


def make_core_inputs(x, y, W1, b1, W2, b2, Wm, bm, Wd, bd, tp=TP, to=TO):
    """Host-side prep: returns (in_maps list of 8 dicts, scale [512])."""
    NSLOT_ = tp + 1
    XCOLS_ = NSLOT_ * B
    x = np.asarray(x, np.float32)
    y = np.asarray(y, np.float32)
    W1 = np.asarray(W1, np.float32)
    b1 = np.asarray(b1, np.float32)
    W2 = np.asarray(W2, np.float32)
    b2 = np.asarray(b2, np.float32)
    Wm = np.asarray(Wm, np.float32)
    bm = np.asarray(bm, np.float32)
    Wd = np.asarray(Wd, np.float32)
    bd = np.asarray(bd, np.float32)

    scale = 1.0 + np.mean(y[:, 0:to, 0], axis=1)       # [512]
    y_sc = y[:, 0:to, 0] / scale[:, None]              # [512, to]

    b1a = b1.copy()
    b1a[2 * H:3 * H] += 1.0                             # forget-gate +1
    b2a = b2.copy()
    b2a[2 * H:3 * H] += 1.0

    # row layout: 0 = y/m, 1:64 = x[0:63], 64 = ones/bias (disp storage),
    # 65 = x[63]  (rows 0 and 64 must sit at legal engine partition bases)
    w1c0 = np.empty((66, G), np.float32)
    w1c0[0] = W1[F]                                     # y/m weight row
    w1c0[1:64] = W1[0:F - 1]                            # x weight rows 0..62
    w1c0[64] = b1a                                      # bias row (ones input)
    w1c0[65] = W1[F - 1]                                # x weight row 63

    w1h = np.ascontiguousarray(
        W1[F + 1:].reshape(4, 128, G).transpose(1, 0, 2).reshape(128, 4 * G))
    w2 = np.ascontiguousarray(
        W2.reshape(8, 128, G).transpose(1, 0, 2).reshape(128, 8 * G))

    wmd = np.zeros((128, 4, 64), np.float32)
    wmd[:, :, 0] = Wm[:, 0].reshape(4, 128).T
    wmd[:, :, 32] = Wd[:, 0].reshape(4, 128).T
    wmd = np.ascontiguousarray(wmd.reshape(128, 4 * 64))

    b2row = np.ascontiguousarray(b2a.reshape(1, G))
    bmd = np.zeros((33, 1), np.float32)
    bmd[0, 0] = bm[0]
    bmd[32, 0] = bd[0]
    identity = np.eye(64, dtype=np.float32)

    in_maps = []
    for c in range(NC):
        bs = slice(c * B, (c + 1) * B)
        xyf = np.zeros((66, NSLOT_, B), np.float32)
        xyf[0, 1:to, :] = y_sc[bs, 0:to - 1].T          # shifted y feed
        xt = x[bs].transpose(2, 1, 0)                   # [f, t, b]
        xyf[1:64, 0:tp, :] = xt[0:F - 1, 0:tp]          # x rows 0..62
        xyf[65, 0:tp, :] = xt[F - 1, 0:tp]              # x row 63
        xyf[64, :, :] = 1.0                             # ones / bias row
        in_maps.append({
            "xyf_d": np.ascontiguousarray(xyf.reshape(66, XCOLS_)),
            "w1c0_d": w1c0, "w1h_d": w1h, "w2_d": w2, "wmd_d": wmd,
            "b2row_d": b2row, "bmd_d": bmd, "id_d": identity,
        })
    return in_maps, scale


def postprocess(results, scale, tp=TP, to=TO):
    """results: list of 8 dicts with out_d [2, (tp-to)*64] -> [512, tp-to, 2]."""
    out = np.empty((B_FULL, tp - to, 2), np.float32)
    for c in range(NC):
        r = results[c]["out_d"]
        mean_tb = r[0].reshape(tp - to, B)              # [t, b]
        dpre_tb = r[1].reshape(tp - to, B)
        bs = slice(c * B, (c + 1) * B)
        sc = scale[bs]
        out[bs, :, 0] = (mean_tb * sc[None, :]).T
        disp = np.logaddexp(dpre_tb, 0.0)               # softplus
        out[bs, :, 1] = (disp * np.sqrt(sc)[None, :]).T
    return out


def kernel(x, y, W1, b1, W2, b2, Wm, bm, Wd, bd):
    in_maps, scale = make_core_inputs(x, y, W1, b1, W2, b2, Wm, bm, Wd, bd)
    nc = _get_program()
    res = bass_utils.run_bass_kernel_spmd(nc, in_maps, core_ids=list(range(NC)))
    return postprocess(res.results, scale)
